# revision 8
# baseline (speedup 1.0000x reference)
"""CrossAttention Trainium2 kernel, v2.

Data-parallel over batch across 8 NeuronCores (4 batches each).

Key layout/scheduling choices (probe-driven):
- Narrow-N matmul atoms (N=128) for kproj/vproj/AV: rotating-weight matmuls
  run ~70ns at N=128 but ~695ns at N=512 on this part (ldweights doesn't
  hide behind wide streams).
- QK is weight-stationary (lhsT = q head block reused across all n chunks)
  so it uses N=512 atoms.
- Softmax skips max-subtraction (logits bounded ~|6|); the additive mask is
  folded in multiplicatively via host-precomputed exp(mask), fused with the
  rowsum on DVE (scalar_tensor_tensor accum_out).
- PSUM->SBUF drains split across engines: Act drains kproj, DVE drains vproj.
"""
import os
import sys

sys.path.insert(0, "/opt/trn_rl_repo")

import numpy as np
import ml_dtypes

import concourse.bacc as bacc
import concourse.mybir as mybir
import concourse.tile as tile

BF = ml_dtypes.bfloat16

B, QN, N, DIM, HEADS, HD = 32, 128, 4096, 512, 8, 64
SCALE = HD ** -0.5
NCORES = 8
BL = B // NCORES  # batches per core
NT = N // 128     # 32 token tiles

f32 = mybir.dt.float32
bf16 = mybir.dt.bfloat16
ADD = mybir.AluOpType.add
MULT = mybir.AluOpType.mult
EXP = mybir.ActivationFunctionType.Exp

_built = {}
_runner = {}

KW = 128   # kproj/vproj matmul free width
QKW = 512  # QK matmul free width (weight-stationary)


def _emit(nc, reps=1):
    kvT_d = nc.dram_tensor("kvT", [BL, 4, 128, N], bf16, kind="ExternalInput").ap()
    qT_d = nc.dram_tensor("qT", [4, 128, BL * QN], bf16, kind="ExternalInput").ap()
    m_d = nc.dram_tensor("m", [BL, QN, N], bf16, kind="ExternalInput").ap()
    wkvT_d = nc.dram_tensor("wkvT", [4, 128, 2 * DIM], bf16, kind="ExternalInput").ap()
    wqT_d = nc.dram_tensor("wqT", [4, 128, DIM], bf16, kind="ExternalInput").ap()
    wpT_d = nc.dram_tensor("wpT", [4, 128, DIM], bf16, kind="ExternalInput").ap()
    bias_d = nc.dram_tensor("biasb", [128, DIM], f32, kind="ExternalInput").ap()
    out_d = nc.dram_tensor("out", [BL, QN, DIM], f32, kind="ExternalOutput").ap()

    with tile.TileContext(nc) as tc:
        with (
            tc.tile_pool(name="wpool", bufs=1) as wpool,
            tc.tile_pool(name="kvtp", bufs=4) as kvtp,
            tc.tile_pool(name="mp", bufs=1) as mp,
            tc.tile_pool(name="ktp", bufs=4) as ktp,
            tc.tile_pool(name="vp", bufs=2) as vp,
            tc.tile_pool(name="pp", bufs=2) as pp,
            tc.tile_pool(name="ptp", bufs=2) as ptp,
            tc.tile_pool(name="xp", bufs=5) as xp,
            tc.tile_pool(name="rp", bufs=2) as rp,
            tc.tile_pool(name="outp", bufs=2) as outp,
            tc.tile_pool(name="ps", bufs=2, space="PSUM") as psp,
            tc.tile_pool(name="psx", bufs=4, space="PSUM") as psxp,
        ):
            # ---- persistent weights + consts ----
            wkvT, wqT, wpT, qT = [], [], [], []
            for t in range(4):
                wk = wpool.tile([128, 2 * DIM], bf16, name=f"wkvT{t}")
                nc.sync.dma_start(out=wk, in_=wkvT_d[t])
                wkvT.append(wk)
                wq = wpool.tile([128, DIM], bf16, name=f"wqT{t}")
                nc.sync.dma_start(out=wq, in_=wqT_d[t])
                wqT.append(wq)
                wp = wpool.tile([128, DIM], bf16, name=f"wpT{t}")
                nc.sync.dma_start(out=wp, in_=wpT_d[t])
                wpT.append(wp)
                qt = wpool.tile([128, BL * QN], bf16, name=f"qT{t}")
                nc.sync.dma_start(out=qt, in_=qT_d[t])
                qT.append(qt)
            bias_sb = wpool.tile([128, DIM], f32, name="bias_sb")
            nc.sync.dma_start(out=bias_sb, in_=bias_d)

            # ---- q projection: qhT[pr] = [c_out 128, (b q) 512] bf16 ----
            qhT = []
            for pr in range(4):
                ps_q = psp.tile([128, 1024], f32, name="ps_t")
                for ch in range(4):
                    for ci in range(4):
                        nc.tensor.matmul(
                            ps_q[:, ch * KW:(ch + 1) * KW],
                            wqT[ci][:, pr * 128:(pr + 1) * 128],
                            qT[ci][:, ch * KW:(ch + 1) * KW],
                            start=(ci == 0),
                            stop=(ci == 3),
                        )
                qh = wpool.tile([128, BL * QN], bf16, name=f"qhT{pr}")
                nc.scalar.copy(qh, ps_q[:, 0:512])
                qhT.append(qh)

            for rep in range(reps):
              for b in range(BL):
                # ---- load kvT (features x tokens) + mask ----
                kvt = []
                for t in range(4):
                    kv_t = kvtp.tile([128, N], bf16, name="kv_t")
                    nc.gpsimd.dma_start(out=kv_t, in_=kvT_d[b, t])
                    kvt.append(kv_t)
                m_t = mp.tile([128, N], bf16, name="m_t")
                nc.gpsimd.dma_start(out=m_t, in_=m_d[b])

                v_all = vp.tile([128, NT, DIM], bf16, name="v_all")

                # ---- kproj (Act drains) + vproj (DVE drains), interleaved ----
                kt = []
                for pr in range(4):
                    k_t = ktp.tile([128, N], bf16, name="k_t")
                    # kproj: 4 psum tiles of 1024 cols each
                    for g in range(4):
                        ps_k = psp.tile([128, 1024], f32, name="ps_t")
                        for ch in range(8):
                            for ci in range(4):
                                c0 = g * 1024 + ch * KW
                                nc.tensor.matmul(
                                    ps_k[:, ch * KW:(ch + 1) * KW],
                                    wkvT[ci][:, pr * 128:(pr + 1) * 128],
                                    kvt[ci][:, c0:c0 + KW],
                                    start=(ci == 0),
                                    stop=(ci == 3),
                                )
                        nc.scalar.copy(k_t[:, g * 1024:(g + 1) * 1024], ps_k)
                    kt.append(k_t)
                    # vproj for 8 token tiles (2 per psum tile)
                    for g in range(4):
                        ps_v = psp.tile([128, 1024], f32, name="ps_t")
                        for t2 in range(2):
                            tt = pr * 8 + g * 2 + t2
                            for fh in range(4):
                                for ci in range(4):
                                    nc.tensor.matmul(
                                        ps_v[:, t2 * 512 + fh * KW:
                                             t2 * 512 + (fh + 1) * KW],
                                        kvt[ci][:, tt * 128:(tt + 1) * 128],
                                        wkvT[ci][:, DIM + fh * KW:
                                                 DIM + (fh + 1) * KW],
                                        start=(ci == 0),
                                        stop=(ci == 3),
                                    )
                        tt0 = pr * 8 + g * 2
                        nc.vector.tensor_copy(
                            v_all[:, tt0:tt0 + 2, :],
                            ps_v.rearrange("p (t f) -> p t f", t=2),
                        )

                # ---- attention per head pair ----
                xT = []
                for pr in range(4):
                    x_t = xp.tile([128, QN], bf16, name="x_t")
                    p_sb = []
                    for hh in range(2):
                        p_h = pp.tile([128, N], bf16, name="p_h")
                        p_sb.append(p_h)
                    # QK (weight-stationary, N=512) + exp
                    for hh in range(2):
                        r0 = hh * 64
                        for g in range(4):
                            ps_s = psp.tile([128, 1024], f32, name="ps_t")
                            for half in range(2):
                                n0 = g * 1024 + half * QKW
                                nc.tensor.matmul(
                                    ps_s[:, half * QKW:(half + 1) * QKW],
                                    qhT[pr][r0:r0 + 64, b * QN:(b + 1) * QN],
                                    kt[pr][r0:r0 + 64, n0:n0 + QKW],
                                    start=True,
                                    stop=True,
                                    tile_position=(r0, 0),
                                )
                            nc.scalar.activation(
                                p_sb[hh][:, g * 1024:(g + 1) * 1024], ps_s, EXP
                            )
                    # mask multiply + rowsum + normalize + transpose per head
                    pt_sb = []
                    for hh in range(2):
                        rowsum = rp.tile([128, 1], f32, name="rowsum")
                        nc.vector.scalar_tensor_tensor(
                            out=p_sb[hh],
                            in0=p_sb[hh],
                            scalar=1.0,
                            in1=m_t,
                            op0=MULT,
                            op1=MULT,
                            accum_out=rowsum,
                        )
                        recip = rp.tile([128, 1], f32, name="recip")
                        nc.vector.reciprocal(recip, rowsum)
                        nc.vector.tensor_scalar_mul(p_sb[hh], p_sb[hh], recip)
                        pt_h = ptp.tile([128, NT, 128], bf16, name="pt_h")
                        nc.sync.dma_start_transpose(pt_h, p_sb[hh])
                        pt_sb.append(pt_h)
                    # AV, column-packed across the 2 heads
                    ps_x = psxp.tile([128, QN], f32, name="ps_x")
                    for i in range(NT):
                        nc.tensor.matmul(
                            ps_x[0:64, :],
                            v_all[:, i, 2 * pr * 64:(2 * pr + 1) * 64],
                            pt_sb[0][:, i, :],
                            start=(i == 0),
                            stop=(i == NT - 1),
                            tile_position=(0, 0),
                            skip_group_check=True,
                        )
                        nc.tensor.matmul(
                            ps_x[64:128, :],
                            v_all[:, i, (2 * pr + 1) * 64:(2 * pr + 2) * 64],
                            pt_sb[1][:, i, :],
                            start=(i == 0),
                            stop=(i == NT - 1),
                            tile_position=(0, 64),
                            skip_group_check=True,
                        )
                    nc.vector.tensor_copy(x_t, ps_x)
                    xT.append(x_t)

                # ---- output projection ----
                ps_o = psp.tile([128, 1024], f32, name="ps_t")
                for fh in range(4):
                    for pr in range(4):
                        nc.tensor.matmul(
                            ps_o[:, fh * KW:(fh + 1) * KW],
                            xT[pr],
                            wpT[pr][:, fh * KW:(fh + 1) * KW],
                            start=(pr == 0),
                            stop=(pr == 3),
                        )
                out_sb = outp.tile([128, DIM], f32, name="out_sb")
                nc.vector.tensor_tensor(
                    out=out_sb, in0=ps_o[:, 0:512], in1=bias_sb, op=ADD
                )
                nc.gpsimd.dma_start(out=out_d[b], in_=out_sb)
    return nc


def build(reps=1):
    if reps not in _built:
        nc = bacc.Bacc(
            "TRN2", target_bir_lowering=False, debug=False, num_devices=NCORES
        )
        _emit(nc, reps)
        nc.compile()
        _built[reps] = nc
    return _built[reps]


def prep_inputs(q, kv, key_mask, Wq, Wkv, Wproj, bproj):
    """Host-side shard + layout prep. Returns per-core in_maps."""
    q = np.asarray(q, dtype=np.float32)
    kv = np.asarray(kv, dtype=np.float32)
    key_mask = np.asarray(key_mask, dtype=np.float32)
    wkvT = np.ascontiguousarray(np.asarray(Wkv, np.float32).T).astype(BF)
    wkvT = wkvT.reshape(4, 128, 2 * DIM)
    wqT = np.ascontiguousarray((np.asarray(Wq, np.float32) * SCALE).T).astype(BF)
    wqT = wqT.reshape(4, 128, DIM)
    wpT = np.ascontiguousarray(np.asarray(Wproj, np.float32).T).astype(BF)
    wpT = wpT.reshape(4, 128, DIM)
    biasb = np.ascontiguousarray(
        np.broadcast_to(np.asarray(bproj, np.float32), (128, DIM))
    )

    kv_bf = kv.astype(BF)
    m_bf = np.exp(key_mask).astype(BF)

    in_maps = []
    for c in range(NCORES):
        sl = slice(c * BL, (c + 1) * BL)
        kvT = np.ascontiguousarray(kv_bf[sl].transpose(0, 2, 1)).reshape(
            BL, 4, 128, N
        )
        q_loc = q[sl].astype(BF)  # [BL, QN, DIM]
        qT = np.ascontiguousarray(q_loc.transpose(2, 0, 1)).reshape(4, 128, BL * QN)
        in_maps.append(
            {
                "kvT": kvT,
                "qT": qT,
                "m": np.ascontiguousarray(m_bf[sl]),
                "wkvT": wkvT,
                "wqT": wqT,
                "wpT": wpT,
                "biasb": biasb,
            }
        )
    return in_maps


class Runner:
    """Jitted SPMD executor with device-resident inputs for repeat timing."""

    def __init__(self, reps=1):
        import jax
        from concourse.bass2jax import (
            _bass_exec_p,
            install_neuronx_cc_hook,
            partition_id_tensor,
        )
        from jax.experimental.shard_map import shard_map
        from jax.sharding import Mesh, PartitionSpec

        self.jax = jax
        nc = build(reps)
        install_neuronx_cc_hook()
        pname = nc.partition_id_tensor.name if nc.partition_id_tensor else None
        in_names, out_names, out_avals = [], [], []
        for alloc in nc.m.functions[0].allocations:
            if not isinstance(alloc, mybir.MemoryLocationSet):
                continue
            name = alloc.memorylocations[0].name
            if alloc.kind == "ExternalInput":
                if name != pname:
                    in_names.append(name)
            elif alloc.kind == "ExternalOutput":
                out_names.append(name)
                out_avals.append(
                    jax.core.ShapedArray(
                        tuple(alloc.tensor_shape), mybir.dt.np(alloc.dtype)
                    )
                )
        self.in_names = list(in_names)
        self.out_names = out_names
        self.out_avals = out_avals
        n_params = len(in_names)
        all_names = in_names + out_names
        if pname is not None:
            all_names = all_names + [pname]
        donate = tuple(range(n_params, n_params + len(out_names)))

        def _body(*args):
            operands = list(args)
            if pname is not None:
                operands.append(partition_id_tensor())
            outs = _bass_exec_p.bind(
                *operands,
                out_avals=tuple(out_avals),
                in_names=tuple(all_names),
                out_names=tuple(out_names),
                lowering_input_output_aliases=(),
                sim_require_finite=True,
                sim_require_nnan=True,
                nc=nc,
            )
            return tuple(outs)

        devices = jax.devices()[:NCORES]
        self.mesh = Mesh(np.asarray(devices), ("core",))
        self.pspec = PartitionSpec("core")
        in_specs = (self.pspec,) * (n_params + len(out_names))
        out_specs = (self.pspec,) * len(out_names)
        self.fn = jax.jit(
            shard_map(
                _body,
                mesh=self.mesh,
                in_specs=in_specs,
                out_specs=out_specs,
                check_rep=False,
            ),
            donate_argnums=donate,
            keep_unused=True,
        )

    def put_inputs(self, in_maps):
        """Concat per-core inputs on axis 0 and move to devices (sharded)."""
        from jax.sharding import NamedSharding

        sh = NamedSharding(self.mesh, self.pspec)
        dev = []
        for name in self.in_names:
            cat = np.concatenate([m[name] for m in in_maps], axis=0)
            dev.append(self.jax.device_put(cat, sh))
        return dev

    def zeros(self):
        from jax.sharding import NamedSharding

        sh = NamedSharding(self.mesh, self.pspec)
        return [
            self.jax.device_put(
                np.zeros((NCORES * a.shape[0], *a.shape[1:]), a.dtype), sh
            )
            for a in self.out_avals
        ]

    def run(self, dev_inputs, zeros=None):
        if zeros is None:
            zeros = self.zeros()
        outs = self.fn(*dev_inputs, *zeros)
        self.jax.block_until_ready(outs)
        return outs


def get_runner(reps=1):
    if reps not in _runner:
        _runner[reps] = Runner(reps)
    return _runner[reps]


def kernel(q, kv, key_mask, Wq, Wkv, Wproj, bproj):
    r = get_runner()
    in_maps = prep_inputs(q, kv, key_mask, Wq, Wkv, Wproj, bproj)
    dev = r.put_inputs(in_maps)
    outs = r.run(dev)
    out = np.asarray(outs[0]).reshape(NCORES, BL, QN, DIM).reshape(B, QN, DIM)
    return out.astype(np.float32)
